# revision 1
# baseline (speedup 1.0000x reference)
"""CTC batch cost (Keras convention) on 8 Trainium2 NeuronCores.

Raw-Bass static pipeline (no Tile): explicit engine streams + semaphores.
Raw mode emits semaphore waits as standalone sequencer instructions, which
avoids the 1-wait limit of embedded sync on matmul/DMA pseudo-instructions.

Per core (32 batch rows):
  - Host uploads log(y_pred+1e-7) packed with one-hot gather matrices
    [b, C, T+S], skewed transition masks, and a +32 partition permutation.
  - Gather: PE one-hot matmuls produce logP [S, T] per b (exact gather);
    ScalarE copies PSUM->SBUF; DMAs scatter into a skewed slab with
    partitions = (b, time-segment j), free dim = wavefront cells.
  - Viterbi pass (log space, overflow-immune): 100-cycle wavefront, per
    cycle one scalar_tensor_tensor (add/max) + one tensor_tensor_scan
    (max, add) on DVE; cross-segment halos via PE permutation matmul +
    ScalarE copies.
  - Per-segment max-path levels via strided max-reduces -> per-partition
    exp biases (measured rates + compile-time khat tilt).
  - ScalarE exp -> scaled linear slab; forward pass = same wavefront with
    (mult/add) + scan (add, mult); state bounded within ~e+-50.
  - loss = -(log(alpha_T[S-1]+alpha_T[S-2]) + Vstar_T + 128*sum(khat)).

The program is input-value-independent; built/compiled once, reused.
"""

from contextlib import ExitStack

import numpy as np

import concourse.bass as bass
import concourse.mybir as mybir
from concourse.bass_utils import run_bass_kernel_spmd

F32 = mybir.dt.float32
AF = mybir.ActivationFunctionType
OP = mybir.AluOpType
NEG = -1e30
EPS = 1e-7

B, T, C, U = 256, 512, 128, 48
S = 2 * U + 1          # 97
BLANK = C - 1
NCORES = 8
BPC = B // NCORES      # 32
NSEG = 4
SEG = T // NSEG        # 128
W = SEG + 1            # cell width (halo slot + 128 values)
NCYC = S + NSEG - 1    # 100
LEAD = 2
KHAT = (0.252, 0.137, 0.137, 0.137)
KSUM = SEG * sum(KHAT)
GRP = 8                # b per mega-DMA
NGRP = BPC // GRP      # 4
PSLAB = NCYC * SEG     # 12800
VSLAB = (NCYC + LEAD) * W

_cache = {}


def _cb(s0):
    return (s0 + LEAD) * W


def build_program():
    nc = bass.Bass()
    ygpack = nc.declare_dram_parameter("ygpack", [BPC, C, T + S], F32, isOutput=False)
    mlog = nc.declare_dram_parameter("mlog", [128, NCYC], F32, isOutput=False)
    mlin = nc.declare_dram_parameter("mlin", [128, NCYC], F32, isOutput=False)
    perm = nc.declare_dram_parameter("perm", [128, 128], F32, isOutput=False)
    paug = nc.declare_dram_parameter("paug", [128, 128], F32, isOutput=False)
    negc = nc.declare_dram_parameter("negc", [128, 1], F32, isOutput=False)
    loss = nc.declare_dram_parameter("loss", [BPC, 1], F32, isOutput=True)

    ctx = ExitStack()

    def sbuf(shape, name):
        return ctx.enter_context(nc.sbuf_tensor(name, shape, F32))

    def psumt(shape, name):
        return ctx.enter_context(nc.psum_tensor(name, shape, F32))

    def semp(name):
        return ctx.enter_context(nc.semaphore(name))

    with ctx:
        permst = sbuf([128, 128], "permst")
        paugt = sbuf([128, 128], "paugt")
        negct = sbuf([128, 1], "negct")
        mlogt = sbuf([128, NCYC], "mlogt")
        mlint = sbuf([128, NCYC], "mlint")
        ygt = [sbuf([C, GRP * (T + S)], f"ygt{i}") for i in range(2)]
        stg = [sbuf([S, T], f"stg{i}") for i in range(4)]
        pslab = sbuf([128, PSLAB], "pslab")
        phslab = sbuf([128, PSLAB], "phslab")
        vslab = sbuf([128, VSLAB], "vslab")
        uu = [sbuf([128, SEG], f"u{i}") for i in range(2)]
        atile = sbuf([128, 1], "atile")
        ctile = sbuf([128, 1], "ctile")
        btile = sbuf([128, 1], "btile")
        khat_t = sbuf([128, 1], "khat_t")
        d1 = sbuf([128, 1], "d1")
        bias_t = sbuf([128, 1], "bias_t")
        rout = [sbuf([128, 1], f"rout{j}") for j in range(NSEG)]
        vt = sbuf([128, 1], "vt")
        lt = sbuf([128, 1], "lt")
        st = sbuf([128, 1], "st")
        lossT = sbuf([128, 1], "lossT")

        ps = [psumt([S, T], f"ps{i}") for i in range(2)]
        ph = [psumt([128, 1], f"ph{i}") for i in range(2)]
        bps = psumt([128, 1], "bps")

        sem_c = semp("sem_c")
        sem_y = [semp("sem_y0"), semp("sem_y1")]
        sem_sk = [semp(f"sem_sk{i}") for i in range(4)]  # per stg-slot skews
        sem_v = semp("sem_v")
        sem_a = semp("sem_a")
        sem_p = semp("sem_p")
        sem_o = semp("sem_o")

        # ---- planned semaphore tick values ----
        # PE: 32 gather mms (1..32), viterbi perms (33..131), btile perm
        # (132), linear perms (133..231)
        p_mm = {b: b + 1 for b in range(BPC)}
        p_perm_v = {s0: BPC + 1 + s0 for s0 in range(NCYC - 1)}
        p_bperm = BPC + NCYC
        p_perm_l = {s0: p_bperm + 1 + s0 for s0 in range(NCYC - 1)}
        # ACT: stg copies (1..32), viterbi halos (33..131: one inc per
        # cycle after 2nd copy), atile/ctile copies (132..138), btile
        # (139), exp (140), linear halos (141..239), Ln (240), final (241)
        a_cp = {b: b + 1 for b in range(BPC)}
        a_hv = {s0: BPC + 1 + s0 for s0 in range(NCYC - 1)}
        a_abc = BPC + NCYC - 1 + 7
        a_btile = a_abc + 1
        a_exp = a_btile + 1
        a_hl = {s0: a_exp + 1 + s0 for s0 in range(NCYC - 1)}
        a_ln = a_exp + NCYC
        a_fin = a_ln + 1
        # DVE: 6 pslab fake memsets + 3 viterbi init (->9), viterbi scans
        # (10..109), 4 reduces (110..113), d1 (114), bias (115), linear
        # init (116..118), linear scans (119..218), vt (219), st (220)
        v_ms = 9
        v_scan_v = {s0: v_ms + 1 + s0 for s0 in range(NCYC)}
        v_red = {j: v_ms + NCYC + 1 + j for j in range(NSEG)}
        v_bias = v_ms + NCYC + NSEG + 2
        v_init_l = v_bias + 3
        v_scan_l = {s0: v_init_l + 1 + s0 for s0 in range(NCYC)}
        v_vt = v_init_l + NCYC + 1
        v_st = v_vt + 1

        with nc.Block() as block:

            @block.sync
            def _(sync):
                sync.dma_start(permst[:], perm[:]).then_inc(sem_c, 16)
                sync.dma_start(paugt[:], paug[:]).then_inc(sem_c, 16)
                sync.dma_start(negct[:], negc[:]).then_inc(sem_c, 16)
                sync.dma_start(mlogt[:], mlog[:]).then_inc(sem_c, 16)
                sync.dma_start(mlint[:], mlin[:]).then_inc(sem_c, 16)
                ygr = ygpack[:].rearrange("b c w -> c b w")
                p3 = pslab[:].rearrange("p (c w) -> p c w", w=SEG)

                def mega(g):
                    if g >= 2:
                        sync.wait_ge(sem_p, p_mm[(g - 1) * GRP - 1])
                    sync.dma_start(
                        ygt[g % 2][:].rearrange("c (b w) -> c b w", w=T + S),
                        ygr[:, g * GRP:(g + 1) * GRP, :],
                    ).then_inc(sem_y[g % 2], 16)

                def skews(b):
                    sync.wait_ge(sem_a, a_cp[b])
                    for j in range(NSEG):
                        p = b + 32 * j
                        dst = pslab[p:p + 1, j * SEG:j * SEG + S * SEG]
                        sync.dma_start(
                            dst, stg[b % 4][:, j * SEG:(j + 1) * SEG]
                        ).then_inc(sem_sk[b % 4], 16)

                mega(0)
                mega(1)
                sync.wait_ge(sem_v, 6)  # pslab fake-region memsets done
                for b in range(GRP):
                    skews(b)
                mega(2)
                for b in range(GRP, 2 * GRP):
                    skews(b)
                mega(3)
                for b in range(2 * GRP, BPC):
                    skews(b)
                sync.wait_ge(sem_a, a_fin)
                sync.dma_start(loss[:, :], lossT[96:128, :]).then_inc(sem_o, 16)
                sync.wait_ge(sem_o, 16)

            @block.tensor
            def _(tensor):
                for b in range(BPC):
                    g = b // GRP
                    if b % GRP == 0:
                        tensor.wait_ge(sem_y[g % 2], 16 * (g // 2 + 1))
                    if b >= 2:
                        tensor.wait_ge(sem_a, a_cp[b - 2])
                    yg3 = ygt[g % 2][:].rearrange("c (b w) -> c b w", w=T + S)
                    bl = b % GRP
                    nc.tensor.matmul(
                        ps[b % 2][:], lhsT=yg3[:, bl, T:T + S],
                        rhs=yg3[:, bl, 0:T], start=True, stop=True,
                    ).then_inc(sem_p, 1)

                def perms(v_scan, a_h, aug):
                    for s0 in range(NCYC - 1):
                        tensor.wait_ge(sem_v, v_scan[s0])
                        if s0 >= 2:
                            tensor.wait_ge(sem_a, a_h[s0 - 2])
                        if aug:
                            nc.tensor.matmul(
                                ph[s0 % 2][:], lhsT=permst[:],
                                rhs=vslab[:, _cb(s0) + SEG:_cb(s0) + SEG + 1],
                                start=True, stop=False,
                            )
                            nc.tensor.matmul(
                                ph[s0 % 2][:], lhsT=paugt[:], rhs=negct[:],
                                start=False, stop=True,
                            ).then_inc(sem_p, 1)
                        else:
                            nc.tensor.matmul(
                                ph[s0 % 2][:], lhsT=permst[:],
                                rhs=vslab[:, _cb(s0) + SEG:_cb(s0) + SEG + 1],
                                start=True, stop=True,
                            ).then_inc(sem_p, 1)

                tensor.wait_ge(sem_c, 80)
                perms(v_scan_v, a_hv, True)
                tensor.wait_ge(sem_a, a_abc)
                nc.tensor.matmul(bps[:], lhsT=permst[:], rhs=ctile[:],
                                 start=True, stop=True).then_inc(sem_p, 1)
                perms(v_scan_l, a_hl, False)

            @block.scalar
            def _(scalar):
                for b in range(BPC):
                    scalar.wait_ge(sem_p, p_mm[b])
                    if b >= 4:
                        # stg slot b%4 reused: b-4's skew DMAs must be done
                        scalar.wait_ge(sem_sk[b % 4], 16 * 4 * (b // 4))
                    nc.scalar.activation(out=stg[b % 4][:], in_=ps[b % 2][:],
                                         func=AF.Copy).then_inc(sem_a, 1)

                def halos(p_perm):
                    for s0 in range(NCYC - 1):
                        scalar.wait_ge(sem_p, p_perm[s0])
                        nc.scalar.activation(
                            out=vslab[32:64, _cb(s0 + 1):_cb(s0 + 1) + 1],
                            in_=ph[s0 % 2][32:64], func=AF.Copy)
                        nc.scalar.activation(
                            out=vslab[64:128, _cb(s0 + 1):_cb(s0 + 1) + 1],
                            in_=ph[s0 % 2][64:128], func=AF.Copy,
                        ).then_inc(sem_a, 1)

                halos(p_perm_v)
                for j in range(1, NSEG + 1):
                    scalar.wait_ge(sem_v, v_red[j - 1])
                    lo, hi = 32 * (j - 1), 32 * j
                    nc.scalar.activation(out=atile[lo:hi], in_=rout[j - 1][lo:hi],
                                         func=AF.Copy).then_inc(sem_a, 1)
                    if j < NSEG:
                        nc.scalar.activation(out=ctile[lo:hi],
                                             in_=rout[j - 1][lo:hi],
                                             func=AF.Copy).then_inc(sem_a, 1)
                scalar.wait_ge(sem_p, p_bperm)
                nc.scalar.activation(out=btile[:], in_=bps[:],
                                     func=AF.Copy).then_inc(sem_a, 1)
                scalar.wait_ge(sem_v, v_bias)
                for i in range(4):
                    scalar.wait_ge(sem_sk[i], 16 * 4 * (BPC // 4))
                nc.scalar.activation(out=phslab[:], in_=pslab[:], func=AF.Exp,
                                     bias=bias_t[:], scale=1.0).then_inc(sem_a, 1)
                halos(p_perm_l)
                scalar.wait_ge(sem_v, v_vt)
                nc.scalar.activation(out=lt[96:128], in_=vt[96:128],
                                     func=AF.Ln).then_inc(sem_a, 1)
                scalar.wait_ge(sem_v, v_st)
                nc.scalar.activation(out=lossT[96:128], in_=st[96:128],
                                     func=AF.Copy, scale=-1.0,
                                     bias=-KSUM).then_inc(sem_a, 1)

            @block.vector
            def _(vector):
                p3 = pslab[:].rearrange("p (c w) -> p c w", w=SEG)
                v3 = vslab[:].rearrange("p (c w) -> p c w", w=W)
                for j in range(NSEG):
                    if j > 0:
                        nc.vector.memset(p3[32 * j:32 * (j + 1), 0:j, :],
                                         NEG).then_inc(sem_v, 1)
                    if j < NSEG - 1:
                        nc.vector.memset(p3[32 * j:32 * (j + 1), j + S:NCYC, :],
                                         NEG).then_inc(sem_v, 1)

                def init_slab(viterbi, base):
                    z = NEG if viterbi else 0.0
                    nc.vector.memset(vslab[:, 0:LEAD * W], z).then_inc(sem_v, 1)
                    nc.vector.memset(v3[:, LEAD:, 0], z).then_inc(sem_v, 1)
                    vector.drain()
                    nc.vector.memset(vslab[0:32, _cb(0):_cb(0) + 1],
                                     0.0 if viterbi else 1.0).then_inc(sem_v, 1)

                def cycles(viterbi, data_slab, a_h, p_perm):
                    for s0 in range(NCYC):
                        if s0 >= 2:
                            vector.wait_ge(sem_a, a_h[s0 - 2])
                        vector.drain()
                        nc.vector.scalar_tensor_tensor(
                            out=uu[s0 % 2][:],
                            in0=vslab[:, _cb(s0 - 2):_cb(s0 - 2) + SEG],
                            scalar=(mlogt if viterbi else mlint)[:, s0:s0 + 1],
                            in1=vslab[:, _cb(s0 - 1):_cb(s0 - 1) + SEG],
                            op0=OP.add if viterbi else OP.mult,
                            op1=OP.max if viterbi else OP.add,
                        )
                        if s0 >= 1:
                            vector.wait_ge(sem_p, p_perm[s0 - 1])
                        vector.drain()
                        nc.vector.tensor_tensor_scan(
                            out=vslab[:, _cb(s0) + 1:_cb(s0) + 1 + SEG],
                            data0=uu[s0 % 2][:],
                            data1=data_slab[:, s0 * SEG:(s0 + 1) * SEG],
                            initial=(ph[(s0 - 1) % 2][:, 0:1] if s0 >= 1
                                     else vslab[:, _cb(s0):_cb(s0) + 1]),
                            op0=OP.max if viterbi else OP.add,
                            op1=OP.add if viterbi else OP.mult,
                        ).then_inc(sem_v, 1)

                init_slab(True, 6)
                for i in range(4):
                    vector.wait_ge(sem_sk[i], 16 * 4 * (BPC // 4))
                vector.wait_ge(sem_c, 80)
                cycles(True, pslab, a_hv, p_perm_v)
                vector.drain()
                nc.vector.memset(ctile[:], 0.0)
                for j in range(1, NSEG + 1):
                    nc.vector.tensor_reduce(
                        out=rout[j - 1][:],
                        in_=v3[:, (j - 1) + LEAD:(j - 1) + LEAD + S, SEG],
                        axis=mybir.AxisListType.X, op=OP.max,
                    ).then_inc(sem_v, 1)
                for j in range(NSEG):
                    nc.vector.memset(khat_t[32 * j:32 * (j + 1)], KHAT[j])
                vector.wait_ge(sem_a, a_btile)
                nc.vector.tensor_tensor(out=d1[:], in0=atile[:], in1=btile[:],
                                        op=OP.subtract).then_inc(sem_v, 1)
                vector.drain()
                nc.vector.scalar_tensor_tensor(
                    out=bias_t[:], in0=d1[:], scalar=-1.0 / SEG, in1=khat_t[:],
                    op0=OP.mult, op1=OP.subtract).then_inc(sem_v, 1)
                # linear init: wait until all viterbi-state consumers done
                vector.wait_ge(sem_a, a_exp)
                vector.wait_ge(sem_p, p_bperm)
                init_slab(False, 115)
                cycles(False, phslab, a_hl, p_perm_l)
                vector.drain()
                nc.vector.tensor_tensor(
                    out=vt[96:128],
                    in0=vslab[96:128, _cb(S + 1) + SEG:_cb(S + 1) + SEG + 1],
                    in1=vslab[96:128, _cb(S + 2) + SEG:_cb(S + 2) + SEG + 1],
                    op=OP.add).then_inc(sem_v, 1)
                vector.wait_ge(sem_a, a_ln)
                nc.vector.tensor_tensor(out=st[96:128], in0=lt[96:128],
                                        in1=atile[96:128],
                                        op=OP.add).then_inc(sem_v, 1)

    return nc


def host_prep(y_true, y_pred):
    y_true = np.asarray(y_true)
    y_pred = np.asarray(y_pred, dtype=np.float32)
    ext = np.full((B, S), BLANK, dtype=np.int64)
    ext[:, 1::2] = y_true.astype(np.int64)
    sh = np.concatenate([np.full((B, 2), -1, dtype=np.int64), ext[:, :-2]], axis=1)
    m = ((ext != BLANK) & (ext != sh))

    lq = np.log(y_pred + EPS).astype(np.float32)  # [B, T, C]

    in_maps = []
    for k in range(NCORES):
        bs = slice(k * BPC, (k + 1) * BPC)
        lqt = np.transpose(lq[bs], (0, 2, 1))  # [32, C, T]
        g = np.zeros((BPC, C, S), dtype=np.float32)
        eb = ext[bs]
        for b in range(BPC):
            g[b, eb[b], np.arange(S)] = 1.0
        ygp = np.ascontiguousarray(np.concatenate([lqt, g], axis=2))
        mk = m[bs]
        mlogv = np.full((128, NCYC), NEG, dtype=np.float32)
        mlinv = np.zeros((128, NCYC), dtype=np.float32)
        for j in range(NSEG):
            for s0 in range(NCYC):
                s = s0 - j
                if 0 <= s < S:
                    mlogv[32 * j:32 * (j + 1), s0] = np.where(mk[:, s], 0.0, NEG)
                    mlinv[32 * j:32 * (j + 1), s0] = mk[:, s].astype(np.float32)
        permv = np.zeros((128, 128), dtype=np.float32)
        for kk in range(96):
            permv[kk, kk + 32] = 1.0
        paugv = np.zeros((128, 128), dtype=np.float32)
        for kk in range(32):
            paugv[kk, kk] = 1.0
        negcv = np.full((128, 1), NEG, dtype=np.float32)
        in_maps.append({"ygpack": ygp, "mlog": mlogv, "mlin": mlinv,
                        "perm": permv, "paug": paugv, "negc": negcv})
    return in_maps


def _ensure_axon_devices():
    """Best-effort: make sure the axon PJRT devices are visible even if the
    calling process pinned jax_platforms to cpu (the reference needs cpu;
    run_bass_kernel_spmd needs the 8 NeuronCore devices)."""
    import jax
    try:
        devs = jax.devices()
        if len(devs) >= NCORES and all(d.platform != "cpu" for d in devs[:1]):
            return
    except Exception:
        pass
    try:
        jax.config.update("jax_platforms", None)
        jax.devices()
    except Exception:
        pass


def kernel(y_true, y_pred):
    _ensure_axon_devices()
    if "nc" not in _cache:
        _cache["nc"] = build_program()
    nc = _cache["nc"]
    in_maps = host_prep(y_true, y_pred)
    res = run_bass_kernel_spmd(nc, in_maps, list(range(NCORES)))
    out = np.concatenate([np.asarray(res.results[k]["loss"], dtype=np.float32)
                          for k in range(NCORES)], axis=0)
    return out.reshape(B, 1).astype(np.float32)



# revision 6
# speedup vs baseline: 28.6696x; 28.6696x over previous
"""CTC batch cost (Keras convention) on 8 Trainium2 NeuronCores — v2.

Per core (32 batch rows):
  - Host gathers log(y_pred+eps) at extended-label states and uploads it
    directly in the skewed wavefront-slab layout (bf16) via 8 large
    multi-partition DMAs over three engine queues (sync/scalar/pool) —
    replacing v1's one-hot gather matmuls + 128 serialized skew DMAs.
  - Wavefront: partitions = (b, segment j), NSEG=4 x SEG=128; skew K=4
    cells/segment, NCYC = S + K*3 = 109 cells.
  - Pass 1 (Viterbi, f32): odd cells (label states) run DVE
    scalar_tensor_tensor (max-combine) + tensor_tensor_scan(max, add);
    even cells (blanks: no skip transition) need only the scan reading the
    previous cell's window directly.  DVE ops chain via a self-semaphore
    (cheaper than drain).  Cross-segment halos: PE permutation matmuls
    (2 cell boundaries per matmul) + ScalarE PSUM->halo-slot copies,
    running K cells ahead so they stay off the DVE critical path.
  - Rates: one strided max-reduce over cell boundaries -> per-segment
    rises -> per-partition exp biases (compile-time khat tilt).
  - ScalarE exp (chunked) produces the scaled linear slab (bf16); pass 2
    starts after the first chunk.
  - Pass 2 (forward, bf16): same wavefront with (mult,add)/(add,mult).
  - loss = -(Ln(alpha[S-1]+alpha[S-2]) + Vstar_T + SEG*sum(khat)).

The program is input-value-independent; built/compiled once, reused.
"""

from contextlib import ExitStack

import numpy as np

import concourse.bass as bass
import concourse.mybir as mybir
from concourse.bass_utils import run_bass_kernel_spmd

F32 = mybir.dt.float32
BF16 = mybir.dt.bfloat16
AF = mybir.ActivationFunctionType
OP = mybir.AluOpType
NEG = -1e30
EPS = 1e-7

B, T, C, U = 256, 512, 128, 48
S = 2 * U + 1            # 97
BLANK = C - 1
NCORES = 8
BPC = B // NCORES        # 32
NSEG = 4
SEG = T // NSEG          # 128
K = 4                    # wavefront skew (cells) per segment; even
NCYC = S + K * (NSEG - 1)   # 109
W = SEG + 1              # vslab cell: [halo slot | SEG values]
LEAD = 2                 # pad cells in front of vslab
KHAT = (0.252, 0.137, 0.137, 0.137)
KSUM = SEG * sum(KHAT)
NBANK = 4                # rotating PSUM banks for halo matmuls
NEXP = 8                 # exp chunks
NPAIR = 53               # halo matmuls: pair m = cells (2m, 2m+1)
CSPL = 56                # pslab upload chunk split (cell index)

MLOG0 = 128              # const-tensor column offsets
MLIN0 = 128 + NCYC
KH0 = 128 + 2 * NCYC
CW = 128 + 2 * NCYC + 1

_cache = {}


def _cb(c):
    return (c + LEAD) * W


def build_program():
    nc = bass.Bass()
    pj = [nc.declare_dram_parameter(f"pj{j}", [BPC, (NCYC - K * j) * SEG],
                                    BF16, isOutput=False) for j in range(NSEG)]
    consts = nc.declare_dram_parameter("consts", [128, CW], F32, isOutput=False)
    permb = nc.declare_dram_parameter("permb", [128, 128], BF16, isOutput=False)
    loss = nc.declare_dram_parameter("loss", [BPC, 1], F32, isOutput=True)

    eb = [round(i * NCYC / NEXP) for i in range(NEXP + 1)]

    ctx = ExitStack()
    with ctx:
        pslab = ctx.enter_context(nc.sbuf_tensor("pslab", [128, NCYC * SEG], BF16))
        phslab = ctx.enter_context(nc.sbuf_tensor("phslab", [128, NCYC * SEG], BF16))
        v1 = ctx.enter_context(
            nc.sbuf_tensor("v1", [128, (LEAD + NCYC + 1) * W], F32))
        v2 = ctx.enter_context(
            nc.sbuf_tensor("v2", [128, (LEAD + NCYC + 1) * W], BF16))
        cst = ctx.enter_context(nc.sbuf_tensor("cst", [128, CW], F32))
        permbt = ctx.enter_context(nc.sbuf_tensor("permbt", [128, 128], BF16))
        uu = [ctx.enter_context(nc.sbuf_tensor(f"uu{i}", [128, SEG], F32))
              for i in range(2)]
        atile = ctx.enter_context(nc.sbuf_tensor("atile", [128, 1], F32))
        d1 = ctx.enter_context(nc.sbuf_tensor("d1", [128, 1], F32))
        bias_t = ctx.enter_context(nc.sbuf_tensor("bias_t", [128, 1], F32))
        vt = ctx.enter_context(nc.sbuf_tensor("vt", [128, 1], F32))
        lt = ctx.enter_context(nc.sbuf_tensor("lt", [128, 1], F32))
        st = ctx.enter_context(nc.sbuf_tensor("st", [128, 1], F32))
        lossT = ctx.enter_context(nc.sbuf_tensor("lossT", [128, 1], F32))

        ph = [ctx.enter_context(nc.psum_tensor(f"ph{i}", [128, 2], F32))
              for i in range(NBANK)]
        bps = ctx.enter_context(nc.psum_tensor("bps", [128, 1], F32))

        s_v = ctx.enter_context(nc.semaphore("s_v"))
        s_p = ctx.enter_context(nc.semaphore("s_p"))
        s_a = ctx.enter_context(nc.semaphore("s_a"))
        s_e = ctx.enter_context(nc.semaphore("s_e"))
        s_ds = ctx.enter_context(nc.semaphore("s_ds"))
        s_dc = ctx.enter_context(nc.semaphore("s_dc"))
        s_dp = ctx.enter_context(nc.semaphore("s_dp"))
        s_o = ctx.enter_context(nc.semaphore("s_o"))

        # s_a / s_p base counts per pass
        PAIR_BASE = {1: 0, 2: NPAIR}
        MM_BASE = {1: 0, 2: NPAIR + 1}   # +1 = btile matmul between passes

        marks = {}
        scan_done = {}

        with nc.Block() as block:

            @block.vector
            def _(vector):
                sv = 0

                def emit(inst):
                    nonlocal sv
                    inst.then_inc(s_v, 1)
                    sv += 1

                def chain():
                    if sv:
                        vector.wait_ge(s_v, sv)

                def dve_pass(p, vv, slab, mlx, op_u0, op_u1, op_s0, op_s1):
                    for c in range(NCYC):
                        waits = []
                        if c >= K:
                            waits.append((s_a, PAIR_BASE[p] + (c - K) // 2 + 1))
                        if p == 1:
                            if c == 0:
                                waits += [(s_ds, 16), (s_dc, 48), (s_dp, 16)]
                            if c == 3 * K:
                                waits.append((s_ds, 32))
                            if c == CSPL:
                                waits += [(s_ds, 64), (s_dc, 64), (s_dp, 32)]
                        else:
                            need = next(i for i in range(NEXP) if eb[i + 1] > c)
                            waits.append((s_e, need + 1))
                        for sem, val in waits:
                            vector.wait_ge(sem, val)
                        if c % 2 == 1:
                            chain()
                            emit(nc.vector.scalar_tensor_tensor(
                                out=uu[(c // 2) % 2][:],
                                in0=vv[:, _cb(c - 2):_cb(c - 2) + SEG],
                                scalar=cst[:, mlx + c:mlx + c + 1],
                                in1=vv[:, _cb(c - 1):_cb(c - 1) + SEG],
                                op0=op_u0, op1=op_u1))
                            data0 = uu[(c // 2) % 2][:]
                        else:
                            data0 = vv[:, _cb(c - 1):_cb(c - 1) + SEG]
                        chain()
                        emit(nc.vector.tensor_tensor_scan(
                            out=vv[:, _cb(c) + 1:_cb(c) + 1 + SEG],
                            data0=data0,
                            data1=slab[:, c * SEG:(c + 1) * SEG],
                            initial=vv[:, _cb(c):_cb(c) + 1],
                            op0=op_s0, op1=op_s1))
                        scan_done[(p, c)] = sv

                # ---- presets (each inc'd so later readers are ordered) ----
                for j in range(1, NSEG):
                    emit(nc.vector.memset(
                        pslab[32 * j:32 * (j + 1), 0:K * j * SEG], NEG))
                emit(nc.vector.memset(v1[:, 0:LEAD * W], NEG))
                emit(nc.vector.memset(v1[:, _cb(0):_cb(NCYC - 1) + 1:W], NEG))
                chain()
                emit(nc.vector.memset(v1[0:32, _cb(0):_cb(0) + 1], 0.0))

                dve_pass(1, v1, pslab, MLOG0, OP.add, OP.max, OP.max, OP.add)

                chain()
                emit(nc.vector.tensor_reduce(
                    out=atile[:],
                    in_=v1[:, _cb(0) + SEG:_cb(NCYC - 1) + SEG + 1:W],
                    axis=mybir.AxisListType.X, op=OP.max))
                marks["atile"] = sv
                vector.wait_ge(s_p, MM_BASE[2])
                chain()
                emit(nc.vector.tensor_tensor(out=d1[:], in0=atile[:],
                                             in1=bps[:], op=OP.subtract))
                chain()
                emit(nc.vector.scalar_tensor_tensor(
                    out=bias_t[:], in0=d1[:], scalar=-1.0 / SEG,
                    in1=cst[:, KH0:KH0 + 1], op0=OP.mult, op1=OP.subtract))
                marks["bias"] = sv

                emit(nc.vector.memset(v2[:, 0:LEAD * W], 0.0))
                emit(nc.vector.memset(v2[:, _cb(0):_cb(NCYC - 1) + 1:W], 0.0))
                chain()
                emit(nc.vector.memset(v2[0:32, _cb(0):_cb(0) + 1], 1.0))

                dve_pass(2, v2, phslab, MLIN0, OP.mult, OP.add, OP.add, OP.mult)

                chain()
                cS1 = S - 1 + K * 3
                cS2 = S - 2 + K * 3
                emit(nc.vector.tensor_tensor(
                    out=vt[96:128],
                    in0=v2[96:128, _cb(cS2) + SEG:_cb(cS2) + SEG + 1],
                    in1=v2[96:128, _cb(cS1) + SEG:_cb(cS1) + SEG + 1],
                    op=OP.add))
                marks["vt"] = sv
                vector.wait_ge(s_a, 2 * NPAIR + 1)
                chain()
                emit(nc.vector.tensor_tensor(out=st[96:128], in0=lt[96:128],
                                             in1=atile[96:128], op=OP.add))
                marks["st"] = sv

            @block.tensor
            def _(tensor):
                def mms(p, vv, lhs):
                    for m in range(NPAIR):
                        tensor.wait_ge(s_v, scan_done[(p, 2 * m + 1)])
                        if m >= NBANK:
                            tensor.wait_ge(s_a, PAIR_BASE[p] + m - NBANK + 1)
                        elif p == 2:
                            tensor.wait_ge(s_a, NPAIR)
                        c0 = _cb(2 * m) + SEG
                        nc.tensor.matmul(
                            ph[m % NBANK][:], lhsT=lhs,
                            rhs=vv[:, c0:c0 + W + 1:W],
                            start=True, stop=True).then_inc(s_p, 1)

                tensor.wait_ge(s_dc, 32)
                mms(1, v1, cst[:, 0:128])
                tensor.wait_ge(s_v, marks["atile"])
                nc.tensor.matmul(bps[:], lhsT=cst[:, 0:128], rhs=atile[:],
                                 start=True, stop=True).then_inc(s_p, 1)
                mms(2, v2, permbt[:])

            @block.scalar
            def _(scalar):
                scalar.dma_start(cst[:], consts[:]).then_inc(s_dc, 16)
                scalar.wait_ge(s_dc, 16)
                scalar.dma_start(permbt[:], permb[:]).then_inc(s_dc, 16)
                n1 = (CSPL - K) * SEG
                scalar.wait_ge(s_dc, 32)
                scalar.dma_start(pslab[32:64, K * SEG:CSPL * SEG],
                                 pj[1][:, 0:n1]).then_inc(s_dc, 16)
                scalar.wait_ge(s_dc, 48)
                scalar.dma_start(pslab[32:64, CSPL * SEG:NCYC * SEG],
                                 pj[1][:, n1:]).then_inc(s_dc, 16)

                def copies(p, vv):
                    for m in range(NPAIR):
                        scalar.wait_ge(s_p, MM_BASE[p] + m + 1)
                        dc = _cb(2 * m + K)
                        bank = ph[m % NBANK]
                        nc.scalar.activation(
                            out=vv[32:64, dc:dc + W + 1:W],
                            in_=bank[32:64, 0:2], func=AF.Copy)
                        nc.scalar.activation(
                            out=vv[64:128, dc:dc + W + 1:W],
                            in_=bank[64:128, 0:2],
                            func=AF.Copy).then_inc(s_a, 1)

                copies(1, v1)
                scalar.wait_ge(s_v, marks["bias"])
                for i in range(NEXP):
                    c0, c1 = eb[i], eb[i + 1]
                    nc.scalar.activation(
                        out=phslab[:, c0 * SEG:c1 * SEG],
                        in_=pslab[:, c0 * SEG:c1 * SEG],
                        func=AF.Exp, bias=bias_t[:],
                        scale=1.0).then_inc(s_e, 1)
                copies(2, v2)
                scalar.wait_ge(s_v, marks["vt"])
                nc.scalar.activation(out=lt[96:128], in_=vt[96:128],
                                     func=AF.Ln).then_inc(s_a, 1)
                scalar.wait_ge(s_v, marks["st"])
                nc.scalar.activation(out=lossT[96:128], in_=st[96:128],
                                     func=AF.Copy, scale=-1.0,
                                     bias=-KSUM).then_inc(s_a, 1)

            @block.gpsimd
            def _(gp):
                n1 = (CSPL - 2 * K) * SEG
                gp.dma_start(pslab[64:96, 2 * K * SEG:CSPL * SEG],
                             pj[2][:, 0:n1]).then_inc(s_dp, 16)
                gp.wait_ge(s_dp, 16)
                gp.dma_start(pslab[64:96, CSPL * SEG:NCYC * SEG],
                             pj[2][:, n1:]).then_inc(s_dp, 16)

            @block.sync
            def _(sync):
                n0 = CSPL * SEG
                n3 = (CSPL - 3 * K) * SEG
                sync.dma_start(pslab[0:32, 0:n0],
                               pj[0][:, 0:n0]).then_inc(s_ds, 16)
                sync.wait_ge(s_ds, 16)
                sync.dma_start(pslab[96:128, 3 * K * SEG:CSPL * SEG],
                               pj[3][:, 0:n3]).then_inc(s_ds, 16)
                sync.wait_ge(s_ds, 32)
                sync.dma_start(pslab[0:32, n0:],
                               pj[0][:, n0:]).then_inc(s_ds, 16)
                sync.wait_ge(s_ds, 48)
                sync.dma_start(pslab[96:128, CSPL * SEG:NCYC * SEG],
                               pj[3][:, n3:]).then_inc(s_ds, 16)
                sync.wait_ge(s_a, 2 * NPAIR + 2)
                sync.dma_start(loss[:, :], lossT[96:128, :]).then_inc(s_o, 16)
                sync.wait_ge(s_o, 16)

    return nc


def host_prep(y_true, y_pred):
    import ml_dtypes
    y_true = np.asarray(y_true)
    y_pred = np.asarray(y_pred, dtype=np.float32)
    ext = np.full((B, S), BLANK, dtype=np.int64)
    ext[:, 1::2] = y_true.astype(np.int64)
    sh = np.concatenate([np.full((B, 2), -1, dtype=np.int64), ext[:, :-2]],
                        axis=1)
    allow = (ext != BLANK) & (ext != sh)          # [B, S]

    lq = np.log(y_pred + EPS).astype(np.float32)  # [B, T, C]

    permv = np.zeros((128, 128), dtype=np.float32)
    for kk in range(96):
        permv[kk, kk + 32] = 1.0
    khcol = np.zeros(128, np.float32)
    for j in range(NSEG):
        khcol[32 * j:32 * (j + 1)] = KHAT[j]

    in_maps = []
    for kcore in range(NCORES):
        bs = slice(kcore * BPC, (kcore + 1) * BPC)
        lqt = np.transpose(lq[bs], (0, 2, 1))     # [32, C, T]
        lpe = np.take_along_axis(
            lqt, ext[bs][:, :, None].astype(np.int64), axis=1)  # [32, S, T]
        mk = allow[bs]

        m = {}
        for j in range(NSEG):
            ncells = NCYC - K * j
            arr = np.full((BPC, ncells, SEG), NEG, dtype=np.float32)
            arr[:, 0:S, :] = lpe[:, :, j * SEG:(j + 1) * SEG]
            m[f"pj{j}"] = (arr.reshape(BPC, ncells * SEG)
                           .astype(ml_dtypes.bfloat16))

        mlog = np.full((128, NCYC), NEG, dtype=np.float32)
        mlin = np.zeros((128, NCYC), dtype=np.float32)
        for j in range(NSEG):
            rows = slice(32 * j, 32 * (j + 1))
            for c in range(1, NCYC, 2):
                s = c - K * j
                if 0 <= s < S:
                    mlog[rows, c] = np.where(mk[:, s], 0.0, NEG)
                    mlin[rows, c] = mk[:, s].astype(np.float32)

        cstv = np.zeros((128, CW), np.float32)
        cstv[:, 0:128] = permv
        cstv[:, MLOG0:MLOG0 + NCYC] = mlog
        cstv[:, MLIN0:MLIN0 + NCYC] = mlin
        cstv[:, KH0] = khcol
        m["consts"] = cstv
        m["permb"] = permv.astype(ml_dtypes.bfloat16)
        in_maps.append(m)
    return in_maps


def _ensure_axon_devices():
    import jax
    try:
        devs = jax.devices()
        if len(devs) >= NCORES and all(d.platform != "cpu" for d in devs[:1]):
            return
    except Exception:
        pass
    try:
        jax.config.update("jax_platforms", None)
        jax.devices()
    except Exception:
        pass


def kernel(y_true, y_pred):
    _ensure_axon_devices()
    if "nc" not in _cache:
        _cache["nc"] = build_program()
    nc = _cache["nc"]
    in_maps = host_prep(y_true, y_pred)
    res = run_bass_kernel_spmd(nc, in_maps, list(range(NCORES)))
    out = np.concatenate([np.asarray(res.results[k]["loss"], dtype=np.float32)
                          for k in range(NCORES)], axis=0)
    return out.reshape(B, 1).astype(np.float32)


# revision 15
# speedup vs baseline: 34.0558x; 1.1879x over previous
"""CTC batch cost (Keras convention) on 8 Trainium2 NeuronCores — v2.

Per core (32 batch rows):
  - Host gathers log(y_pred+eps) at extended-label states and uploads it
    directly in the skewed wavefront-slab layout (bf16) via chunked
    multi-partition DMAs over three engine queues (sync/scalar/pool) —
    replacing v1's one-hot gather matmuls + 128 serialized skew DMAs.
  - Wavefront: partitions = (b, segment j), NSEG=4 x SEG=128; skew K=6
    cells/segment, NCYC = S + K*3 = 115 cells.
  - Pass 1 (Viterbi, f32): odd cells (label states) run DVE
    scalar_tensor_tensor (max-combine) + tensor_tensor_scan(max, add);
    even cells (blanks: no skip transition) need only the scan reading the
    previous cell's window directly.  DVE ops chain via a self-semaphore
    (cheaper than drain).  Cross-segment halos: PE permutation matmuls
    (4 cell boundaries per matmul) + ScalarE PSUM->halo-slot copies,
    running K cells ahead so they stay off the DVE critical path.
  - Rates: one strided max-reduce over cell boundaries -> per-segment
    rises -> per-partition exp biases (compile-time khat tilt).
  - ScalarE exp (small chunks, interleaved with the halo copies) produces
    the scaled linear slab (bf16); pass 2 starts after the first chunk.
  - Pass 2 (forward, bf16): same wavefront with (mult,add)/(add,mult).
  - loss = -(Ln(alpha[S-1]+alpha[S-2]) + Vstar_T + SEG*sum(khat)).

The program is input-value-independent; built/compiled once, reused.
"""

from contextlib import ExitStack

import numpy as np

import concourse.bass as bass
import concourse.mybir as mybir
from concourse.bass_utils import run_bass_kernel_spmd

F32 = mybir.dt.float32
BF16 = mybir.dt.bfloat16
F8 = mybir.dt.float8e4
NEG8 = -240.0
AF = mybir.ActivationFunctionType
OP = mybir.AluOpType
NEG = -1e30
EPS = 1e-7

B, T, C, U = 256, 512, 128, 48
S = 2 * U + 1            # 97
BLANK = C - 1
NCORES = 8
BPC = B // NCORES        # 32
NSEG = 4
SEG = T // NSEG          # 128
K = 6                    # wavefront skew (cells) per segment; even
NCYC = S + K * (NSEG - 1)   # 115
W = SEG + 1              # vslab cell: [halo slot | SEG values]
LEAD = 2                 # pad cells in front of vslab
KHAT = (0.252, 0.137, 0.137, 0.137)
KSUM = SEG * sum(KHAT)
NBANK = 4                # rotating PSUM banks for halo matmuls
QW = 4                   # cells per halo matmul (quad)
NQUAD = 28               # quads: boundaries of cells 4q..4q+3 (<= 111)
# exp chunk boundaries (cells); ~6-cell chunks, exp runs a chunk ahead
EB = [0, 3, 6, 12, 18, 24, 30, 36, 42, 48, 54, 60, 66, 72, 78, 84, 90, 96,
      102, 109, 115]

MLOG0 = 128              # const-tensor column offsets
MLIN0 = 128 + NCYC
KH0 = 128 + 2 * NCYC
CW = 128 + 2 * NCYC + 1

_cache = {}


def _cb(c):
    return (c + LEAD) * W


def build_program():
    nc = bass.Bass()
    pj = [nc.declare_dram_parameter(f"pj{j}", [BPC, (NCYC - K * j) * SEG],
                                    F8, isOutput=False) for j in range(NSEG)]
    consts = nc.declare_dram_parameter("consts", [128, CW], F32, isOutput=False)
    permb = nc.declare_dram_parameter("permb", [128, 128], BF16, isOutput=False)
    loss = nc.declare_dram_parameter("loss", [BPC, 1], F32, isOutput=True)

    ctx = ExitStack()
    with ctx:
        pslab = ctx.enter_context(nc.sbuf_tensor("pslab", [128, NCYC * SEG], F8))
        phslab = ctx.enter_context(nc.sbuf_tensor("phslab", [128, NCYC * SEG], BF16))
        v1 = ctx.enter_context(
            nc.sbuf_tensor("v1", [128, (LEAD + NCYC + 5) * W], F32))
        v2 = ctx.enter_context(
            nc.sbuf_tensor("v2", [128, (LEAD + NCYC + 5) * W], BF16))
        cst = ctx.enter_context(nc.sbuf_tensor("cst", [128, CW], F32))
        permbt = ctx.enter_context(nc.sbuf_tensor("permbt", [128, 128], BF16))
        uu = [ctx.enter_context(nc.sbuf_tensor(f"uu{i}", [128, SEG], F32))
              for i in range(2)]
        ub = [ctx.enter_context(nc.sbuf_tensor(f"ub{i}", [128, SEG], BF16))
              for i in range(2)]
        atile = ctx.enter_context(nc.sbuf_tensor("atile", [128, 1], F32))
        d1 = ctx.enter_context(nc.sbuf_tensor("d1", [128, 1], F32))
        bias_t = ctx.enter_context(nc.sbuf_tensor("bias_t", [128, 1], F32))
        vt = ctx.enter_context(nc.sbuf_tensor("vt", [128, 1], F32))
        lt = ctx.enter_context(nc.sbuf_tensor("lt", [128, 1], F32))
        st = ctx.enter_context(nc.sbuf_tensor("st", [128, 1], F32))
        lossT = ctx.enter_context(nc.sbuf_tensor("lossT", [128, 1], F32))

        ph = [ctx.enter_context(nc.psum_tensor(f"ph{i}", [128, QW], F32))
              for i in range(NBANK)]
        bps = ctx.enter_context(nc.psum_tensor("bps", [128, 1], F32))

        s_v = ctx.enter_context(nc.semaphore("s_v"))
        s_p = ctx.enter_context(nc.semaphore("s_p"))
        s_a = ctx.enter_context(nc.semaphore("s_a"))
        s_e = ctx.enter_context(nc.semaphore("s_e"))
        s_ds = ctx.enter_context(nc.semaphore("s_ds"))
        s_dc = ctx.enter_context(nc.semaphore("s_dc"))
        s_dp = ctx.enter_context(nc.semaphore("s_dp"))
        s_o = ctx.enter_context(nc.semaphore("s_o"))

        QUAD_BASE = {1: 0, 2: NQUAD}     # s_a base per pass
        MM_BASE = {1: 0, 2: NQUAD + 1}   # s_p base (+1 = btile matmul)

        marks = {}
        scan_done = {}

        def jd(j, c0, c1):
            return (pslab[32 * j:32 * (j + 1), c0 * SEG:c1 * SEG],
                    pj[j][:, (c0 - K * j) * SEG:(c1 - K * j) * SEG])

        with nc.Block() as block:

            @block.vector
            def _(vector):
                sv = 0

                def emit(inst):
                    nonlocal sv
                    inst.then_inc(s_v, 1)
                    sv += 1

                def chain():
                    if sv:
                        vector.wait_ge(s_v, sv)

                def dve_pass(p, vv, slab, mlx, op_u0, op_u1, op_s0, op_s1):
                    uw = uu if p == 1 else ub
                    last_wait = {}
                    for c in range(NCYC):
                        waits = []
                        if c >= K:
                            waits.append(
                                (s_a, QUAD_BASE[p] + (c - K) // QW + 1))
                        if p == 1:
                            gate = {0: [(s_dc, 16), (s_ds, 16)],
                                    6: [(s_dp, 16)], 12: [(s_dp, 32)],
                                    18: [(s_ds, 32)], 24: [(s_ds, 48)],
                                    30: [(s_dp, 48)], 36: [(s_dp, 64)],
                                    42: [(s_ds, 64)], 48: [(s_ds, 80)],
                                    54: [(s_dp, 80)], 60: [(s_dp, 96)],
                                    66: [(s_ds, 96)], 72: [(s_ds, 112)],
                                    78: [(s_dp, 112)], 84: [(s_dp, 128)],
                                    90: [(s_ds, 128)], 96: [(s_ds, 144)]}
                            waits += gate.get(c, [])
                        else:
                            need = next(i for i in range(len(EB) - 1)
                                        if EB[i + 1] > c)
                            waits.append((s_e, need + 1))
                        for sem, val in waits:
                            if last_wait.get(id(sem)) != val:
                                vector.wait_ge(sem, val)
                                last_wait[id(sem)] = val
                        if c % 2 == 1:
                            chain()
                            emit(nc.vector.scalar_tensor_tensor(
                                out=uw[(c // 2) % 2][:],
                                in0=vv[:, _cb(c - 2):_cb(c - 2) + SEG],
                                scalar=cst[:, mlx + c:mlx + c + 1],
                                in1=vv[:, _cb(c - 1):_cb(c - 1) + SEG],
                                op0=op_u0, op1=op_u1))
                            data0 = uw[(c // 2) % 2][:]
                        else:
                            data0 = vv[:, _cb(c - 1):_cb(c - 1) + SEG]
                        chain()
                        emit(nc.vector.tensor_tensor_scan(
                            out=vv[:, _cb(c) + 1:_cb(c) + 1 + SEG],
                            data0=data0,
                            data1=slab[:, c * SEG:(c + 1) * SEG],
                            initial=vv[:, _cb(c):_cb(c) + 1],
                            op0=op_s0, op1=op_s1))
                        scan_done[(p, c)] = sv

                # ---- presets ----
                for j in range(1, NSEG):
                    # fp8 -240 pattern via u32 bitcast (4 elems/lane-cycle)
                    emit(nc.vector.memset(
                        pslab[32 * j:32 * (j + 1),
                              0:K * j * SEG].bitcast(mybir.dt.uint32),
                        0xF7F7F7F7))
                emit(nc.vector.memset(v1[:, 0:LEAD * W], NEG))
                emit(nc.vector.memset(v1[:, _cb(0):_cb(NCYC - 1) + 1:W], NEG))
                chain()
                emit(nc.vector.memset(v1[0:32, _cb(0):_cb(0) + 1], 0.0))

                dve_pass(1, v1, pslab, MLOG0, OP.add, OP.max, OP.max, OP.add)

                chain()
                emit(nc.vector.tensor_reduce(
                    out=atile[:],
                    in_=v1[:, _cb(0) + SEG:_cb(NCYC - 1) + SEG + 1:W],
                    axis=mybir.AxisListType.X, op=OP.max))
                marks["atile"] = sv
                vector.wait_ge(s_p, MM_BASE[2])
                chain()
                emit(nc.vector.tensor_tensor(out=d1[:], in0=atile[:],
                                             in1=bps[:], op=OP.subtract))
                chain()
                emit(nc.vector.scalar_tensor_tensor(
                    out=bias_t[:], in0=d1[:], scalar=-1.0 / SEG,
                    in1=cst[:, KH0:KH0 + 1], op0=OP.mult, op1=OP.subtract))
                marks["bias"] = sv

                emit(nc.vector.memset(v2[:, 0:LEAD * W], 0.0))
                emit(nc.vector.memset(v2[:, _cb(0):_cb(NCYC - 1) + 1:W], 0.0))
                chain()
                emit(nc.vector.memset(v2[0:32, _cb(0):_cb(0) + 1], 1.0))

                dve_pass(2, v2, phslab, MLIN0, OP.mult, OP.add, OP.add,
                         OP.mult)

                chain()
                cS1 = S - 1 + K * 3   # 114
                cS2 = S - 2 + K * 3   # 113
                emit(nc.vector.tensor_tensor(
                    out=vt[96:128],
                    in0=v2[96:128, _cb(cS2) + SEG:_cb(cS2) + SEG + 1],
                    in1=v2[96:128, _cb(cS1) + SEG:_cb(cS1) + SEG + 1],
                    op=OP.add))
                marks["vt"] = sv
                vector.wait_ge(s_a, 2 * NQUAD + 1)
                chain()
                emit(nc.vector.tensor_tensor(out=st[96:128], in0=lt[96:128],
                                             in1=atile[96:128], op=OP.add))
                marks["st"] = sv

            @block.tensor
            def _(tensor):
                def mms(p, vv, lhs):
                    for q in range(NQUAD):
                        tensor.wait_ge(s_v, scan_done[(p, QW * q + QW - 1)])
                        if q >= NBANK:
                            tensor.wait_ge(s_a,
                                           QUAD_BASE[p] + q - NBANK + 1)
                        elif p == 2:
                            tensor.wait_ge(s_a, NQUAD)
                        c0 = _cb(QW * q) + SEG
                        nc.tensor.matmul(
                            ph[q % NBANK][:], lhsT=lhs,
                            rhs=vv[:, c0:c0 + (QW - 1) * W + 1:W],
                            start=True, stop=True).then_inc(s_p, 1)

                tensor.wait_ge(s_dc, 16)
                mms(1, v1, cst[:, 0:128])
                tensor.wait_ge(s_v, marks["atile"])
                nc.tensor.matmul(bps[:], lhsT=cst[:, 0:128], rhs=atile[:],
                                 start=True, stop=True).then_inc(s_p, 1)
                tensor.wait_ge(s_dc, 32)
                mms(2, v2, permbt[:])

            @block.scalar
            def _(scalar):
                scalar.dma_start(cst[:], consts[:]).then_inc(s_dc, 16)
                scalar.wait_ge(s_dc, 16)
                scalar.dma_start(permbt[:], permb[:]).then_inc(s_dc, 16)
                # preload the ln+exp+copy activation table set
                nc.scalar.activation(out=lt[0:1], in_=cst[0:1, KH0:KH0 + 1],
                                     func=AF.Ln)
                nc.scalar.activation(out=st[0:1], in_=cst[0:1, KH0:KH0 + 1],
                                     func=AF.Exp)

                def copyq(p, vv, q):
                    scalar.wait_ge(s_p, MM_BASE[p] + q + 1)
                    dc = _cb(QW * q + K)
                    bank = ph[q % NBANK]
                    nc.scalar.activation(
                        out=vv[32:64, dc:dc + (QW - 1) * W + 1:W],
                        in_=bank[32:64, 0:QW], func=AF.Copy)
                    nc.scalar.activation(
                        out=vv[64:128, dc:dc + (QW - 1) * W + 1:W],
                        in_=bank[64:128, 0:QW],
                        func=AF.Copy).then_inc(s_a, 1)

                for q in range(NQUAD):
                    copyq(1, v1, q)
                scalar.wait_ge(s_v, marks["bias"])

                def expchunk(i):
                    c0, c1 = EB[i], EB[i + 1]
                    nc.scalar.activation(
                        out=phslab[:, c0 * SEG:c1 * SEG],
                        in_=pslab[:, c0 * SEG:c1 * SEG],
                        func=AF.Exp, bias=bias_t[:],
                        scale=1.0).then_inc(s_e, 1)

                nch = len(EB) - 1
                expchunk(0)
                expchunk(1)
                done_q = 0
                for i in range(2, nch):
                    # copies for quads gated by scans strictly before EB[i-1]
                    target = min(NQUAD, max(0, (EB[i - 1] - 1) // QW))
                    for q in range(done_q, target):
                        copyq(2, v2, q)
                    done_q = target
                    expchunk(i)
                for q in range(done_q, NQUAD):
                    copyq(2, v2, q)
                scalar.wait_ge(s_v, marks["vt"])
                nc.scalar.activation(out=lt[96:128], in_=vt[96:128],
                                     func=AF.Ln).then_inc(s_a, 1)
                scalar.wait_ge(s_v, marks["st"])
                nc.scalar.activation(out=lossT[96:128], in_=st[96:128],
                                     func=AF.Copy, scale=-1.0,
                                     bias=-KSUM).then_inc(s_a, 1)

            @block.gpsimd
            def _(gp):
                chunks = [jd(1, 6, 30), jd(2, 12, 36), jd(1, 30, 54),
                          jd(2, 36, 60), jd(1, 54, 78), jd(2, 60, 84),
                          jd(1, 78, NCYC), jd(2, 84, NCYC)]
                for i, (d, sr) in enumerate(chunks):
                    if i:
                        gp.wait_ge(s_dp, 16 * i)
                    gp.dma_start(d, sr).then_inc(s_dp, 16)

            @block.sync
            def _(sync):
                chunks = [jd(0, 0, 24), jd(3, 18, 42), jd(0, 24, 48),
                          jd(3, 42, 66), jd(0, 48, 72), jd(3, 66, 90),
                          jd(0, 72, 96), jd(3, 90, NCYC), jd(0, 96, NCYC)]
                for i, (d, sr) in enumerate(chunks):
                    if i:
                        sync.wait_ge(s_ds, 16 * i)
                    sync.dma_start(d, sr).then_inc(s_ds, 16)
                sync.wait_ge(s_a, 2 * NQUAD + 2)
                sync.dma_start(loss[:, :], lossT[96:128, :]).then_inc(s_o, 16)
                sync.wait_ge(s_o, 16)

    return nc


def host_prep(y_true, y_pred):
    import ml_dtypes
    y_true = np.asarray(y_true)
    y_pred = np.asarray(y_pred, dtype=np.float32)
    ext = np.full((B, S), BLANK, dtype=np.int64)
    ext[:, 1::2] = y_true.astype(np.int64)
    sh = np.concatenate([np.full((B, 2), -1, dtype=np.int64), ext[:, :-2]],
                        axis=1)
    allow = (ext != BLANK) & (ext != sh)          # [B, S]

    lq = np.log(y_pred + EPS).astype(np.float32)  # [B, T, C]

    permv = np.zeros((128, 128), dtype=np.float32)
    for kk in range(96):
        permv[kk, kk + 32] = 1.0
    khcol = np.zeros(128, np.float32)
    for j in range(NSEG):
        khcol[32 * j:32 * (j + 1)] = KHAT[j]

    in_maps = []
    for kcore in range(NCORES):
        bs = slice(kcore * BPC, (kcore + 1) * BPC)
        lqt = np.transpose(lq[bs], (0, 2, 1))     # [32, C, T]
        lpe = np.take_along_axis(
            lqt, ext[bs][:, :, None].astype(np.int64), axis=1)  # [32, S, T]
        mk = allow[bs]

        m = {}
        for j in range(NSEG):
            ncells = NCYC - K * j
            arr = np.full((BPC, ncells, SEG), -240.0, dtype=np.float32)
            arr[:, 0:S, :] = lpe[:, :, j * SEG:(j + 1) * SEG]
            m[f"pj{j}"] = (arr.reshape(BPC, ncells * SEG)
                           .astype(ml_dtypes.float8_e4m3))

        mlog = np.full((128, NCYC), NEG, dtype=np.float32)
        mlin = np.zeros((128, NCYC), dtype=np.float32)
        for j in range(NSEG):
            rows = slice(32 * j, 32 * (j + 1))
            for c in range(1, NCYC, 2):
                s = c - K * j
                if 0 <= s < S:
                    mlog[rows, c] = np.where(mk[:, s], 0.0, NEG)
                    mlin[rows, c] = mk[:, s].astype(np.float32)

        cstv = np.zeros((128, CW), np.float32)
        cstv[:, 0:128] = permv
        cstv[:, MLOG0:MLOG0 + NCYC] = mlog
        cstv[:, MLIN0:MLIN0 + NCYC] = mlin
        cstv[:, KH0] = khcol
        m["consts"] = cstv
        m["permb"] = permv.astype(ml_dtypes.bfloat16)
        in_maps.append(m)
    return in_maps


def _ensure_axon_devices():
    import jax
    try:
        devs = jax.devices()
        if len(devs) >= NCORES and all(d.platform != "cpu" for d in devs[:1]):
            return
    except Exception:
        pass
    try:
        jax.config.update("jax_platforms", None)
        jax.devices()
    except Exception:
        pass


def kernel(y_true, y_pred):
    _ensure_axon_devices()
    if "nc" not in _cache:
        _cache["nc"] = build_program()
    nc = _cache["nc"]
    in_maps = host_prep(y_true, y_pred)
    res = run_bass_kernel_spmd(nc, in_maps, list(range(NCORES)))
    out = np.concatenate([np.asarray(res.results[k]["loss"], dtype=np.float32)
                          for k in range(NCORES)], axis=0)
    return out.reshape(B, 1).astype(np.float32)


# revision 18
# speedup vs baseline: 34.6602x; 1.0177x over previous
"""CTC batch cost (Keras convention) on 8 Trainium2 NeuronCores — v2.

Per core (32 batch rows):
  - Host gathers log(y_pred+eps) at extended-label states and uploads it
    directly in the skewed wavefront-slab layout (fp8 e4m3, abs err
    <=0.5 on log-probs -> ~2e-3 on the loss) via interleaved chunked
    multi-partition DMAs on the sync and pool queues — replacing v1's
    one-hot gather matmuls + 128 serialized single-partition skew DMAs
    (which dominated v1's 2.6 ms runtime).
  - Wavefront: partitions = (b, segment j), NSEG=4 x SEG=128; skew K=6
    cells/segment, NCYC = S + K*3 = 115 cells.
  - Pass 1 (Viterbi, f32): odd cells (label states) run DVE
    scalar_tensor_tensor (max-combine) + tensor_tensor_scan(max, add);
    even cells (blanks: no skip transition) need only the scan reading the
    previous cell's window directly.  DVE ops chain via a self-semaphore
    (cheaper than drain).  Cross-segment halos: PE permutation matmuls
    (4 cell boundaries per matmul) + ScalarE PSUM->halo-slot copies,
    running K cells ahead so they stay off the DVE critical path.
  - Rates: one strided max-reduce over cell boundaries -> per-segment
    rises -> per-partition exp biases (compile-time khat tilt).
  - ScalarE exp (small chunks, running one chunk ahead of consumption,
    interleaved with the halo copies) produces the scaled linear slab
    (bf16); pass 2 starts after the first chunk.
  - Pass 2 (forward, bf16): same wavefront with (mult,add)/(add,mult).
  - loss = -(Ln(alpha[S-1]+alpha[S-2]) + Vstar_T + SEG*sum(khat)).

The program is input-value-independent; built/compiled once, reused.
"""

from contextlib import ExitStack

import numpy as np

import concourse.bass as bass
import concourse.mybir as mybir
from concourse.bass_utils import run_bass_kernel_spmd

F32 = mybir.dt.float32
BF16 = mybir.dt.bfloat16
F8 = mybir.dt.float8e4
NEG8 = -240.0
AF = mybir.ActivationFunctionType
OP = mybir.AluOpType
NEG = -1e30
EPS = 1e-7

B, T, C, U = 256, 512, 128, 48
S = 2 * U + 1            # 97
BLANK = C - 1
NCORES = 8
BPC = B // NCORES        # 32
NSEG = 4
SEG = T // NSEG          # 128
K = 6                    # wavefront skew (cells) per segment; even
NCYC = S + K * (NSEG - 1)   # 115
W = SEG + 1              # vslab cell: [halo slot | SEG values]
LEAD = 2                 # pad cells in front of vslab
KHAT = (0.252, 0.137, 0.137, 0.137)
KSUM = SEG * sum(KHAT)
NBANK = 4                # rotating PSUM banks for halo matmuls
QW = 4                   # cells per halo matmul (quad)
NQUAD = 28               # quads: boundaries of cells 4q..4q+3 (<= 111)
# exp chunk boundaries (cells); ~5-cell chunks, exp runs a chunk ahead
EB = [0, 3, 6, 10, 15, 20, 25, 30, 35, 40, 45, 50, 55, 60, 65, 70, 75, 80,
      85, 90, 95, 100, 105, 110, 115]

MLOG0 = 128              # const-tensor column offsets
MLIN0 = 128 + NCYC
KH0 = 128 + 2 * NCYC
CW = 128 + 2 * NCYC + 1

_cache = {}


def _cb(c):
    return (c + LEAD) * W


def build_program():
    nc = bass.Bass()
    pj = [nc.declare_dram_parameter(f"pj{j}", [BPC, (NCYC - K * j) * SEG],
                                    F8, isOutput=False) for j in range(NSEG)]
    consts = nc.declare_dram_parameter("consts", [128, CW], F32, isOutput=False)
    permb = nc.declare_dram_parameter("permb", [128, 128], BF16, isOutput=False)
    loss = nc.declare_dram_parameter("loss", [BPC, 1], F32, isOutput=True)

    ctx = ExitStack()
    with ctx:
        pslab = ctx.enter_context(nc.sbuf_tensor("pslab", [128, NCYC * SEG], F8))
        phslab = ctx.enter_context(nc.sbuf_tensor("phslab", [128, NCYC * SEG], BF16))
        v1 = ctx.enter_context(
            nc.sbuf_tensor("v1", [128, (LEAD + NCYC + 5) * W], F32))
        v2 = ctx.enter_context(
            nc.sbuf_tensor("v2", [128, (LEAD + NCYC + 5) * W], BF16))
        cst = ctx.enter_context(nc.sbuf_tensor("cst", [128, CW], F32))
        permbt = ctx.enter_context(nc.sbuf_tensor("permbt", [128, 128], BF16))
        uu = [ctx.enter_context(nc.sbuf_tensor(f"uu{i}", [128, SEG], F32))
              for i in range(2)]
        ub = [ctx.enter_context(nc.sbuf_tensor(f"ub{i}", [128, SEG], BF16))
              for i in range(2)]
        atile = ctx.enter_context(nc.sbuf_tensor("atile", [128, 1], F32))
        d1 = ctx.enter_context(nc.sbuf_tensor("d1", [128, 1], F32))
        bias_t = ctx.enter_context(nc.sbuf_tensor("bias_t", [128, 1], F32))
        vt = ctx.enter_context(nc.sbuf_tensor("vt", [128, 1], F32))
        lt = ctx.enter_context(nc.sbuf_tensor("lt", [128, 1], F32))
        st = ctx.enter_context(nc.sbuf_tensor("st", [128, 1], F32))
        lossT = ctx.enter_context(nc.sbuf_tensor("lossT", [128, 1], F32))

        ph = [ctx.enter_context(nc.psum_tensor(f"ph{i}", [128, QW], F32))
              for i in range(NBANK)]
        bps = ctx.enter_context(nc.psum_tensor("bps", [128, 1], F32))

        s_v = ctx.enter_context(nc.semaphore("s_v"))
        s_p = ctx.enter_context(nc.semaphore("s_p"))
        s_a = ctx.enter_context(nc.semaphore("s_a"))
        s_e = ctx.enter_context(nc.semaphore("s_e"))
        s_ds = ctx.enter_context(nc.semaphore("s_ds"))
        s_dc = ctx.enter_context(nc.semaphore("s_dc"))
        s_dp = ctx.enter_context(nc.semaphore("s_dp"))
        s_o = ctx.enter_context(nc.semaphore("s_o"))

        QUAD_BASE = {1: 0, 2: NQUAD}     # s_a base per pass
        MM_BASE = {1: 0, 2: NQUAD + 1}   # s_p base (+1 = btile matmul)

        marks = {}
        scan_done = {}

        def jd(j, c0, c1):
            return (pslab[32 * j:32 * (j + 1), c0 * SEG:c1 * SEG],
                    pj[j][:, (c0 - K * j) * SEG:(c1 - K * j) * SEG])

        with nc.Block() as block:

            @block.vector
            def _(vector):
                sv = 0

                def emit(inst):
                    nonlocal sv
                    inst.then_inc(s_v, 1)
                    sv += 1

                def chain():
                    if sv:
                        vector.wait_ge(s_v, sv)

                def dve_pass(p, vv, slab, mlx, op_u0, op_u1, op_s0, op_s1):
                    uw = uu if p == 1 else ub
                    last_wait = {}
                    for c in range(NCYC):
                        waits = []
                        if c >= K:
                            waits.append(
                                (s_a, QUAD_BASE[p] + (c - K) // QW + 1))
                        if p == 1:
                            gate = {0: [(s_dc, 16), (s_ds, 16)],
                                    6: [(s_dp, 16)], 12: [(s_dp, 32)],
                                    18: [(s_ds, 32)], 24: [(s_ds, 48)],
                                    30: [(s_dp, 48)], 36: [(s_dp, 64)],
                                    42: [(s_ds, 64)], 48: [(s_ds, 80)],
                                    54: [(s_dp, 80)], 60: [(s_dp, 96)],
                                    66: [(s_ds, 96)], 72: [(s_ds, 112)],
                                    78: [(s_dp, 112)], 84: [(s_dp, 128)],
                                    90: [(s_ds, 128)], 96: [(s_ds, 144)]}
                            waits += gate.get(c, [])
                        else:
                            need = next(i for i in range(len(EB) - 1)
                                        if EB[i + 1] > c)
                            waits.append((s_e, need + 1))
                        for sem, val in waits:
                            if last_wait.get(id(sem)) != val:
                                vector.wait_ge(sem, val)
                                last_wait[id(sem)] = val
                        if c % 2 == 1:
                            chain()
                            emit(nc.vector.scalar_tensor_tensor(
                                out=uw[(c // 2) % 2][:],
                                in0=vv[:, _cb(c - 2):_cb(c - 2) + SEG],
                                scalar=cst[:, mlx + c:mlx + c + 1],
                                in1=vv[:, _cb(c - 1):_cb(c - 1) + SEG],
                                op0=op_u0, op1=op_u1))
                            data0 = uw[(c // 2) % 2][:]
                        else:
                            data0 = vv[:, _cb(c - 1):_cb(c - 1) + SEG]
                        chain()
                        emit(nc.vector.tensor_tensor_scan(
                            out=vv[:, _cb(c) + 1:_cb(c) + 1 + SEG],
                            data0=data0,
                            data1=slab[:, c * SEG:(c + 1) * SEG],
                            initial=vv[:, _cb(c):_cb(c) + 1],
                            op0=op_s0, op1=op_s1))
                        scan_done[(p, c)] = sv

                # ---- presets ----
                for j in range(1, NSEG):
                    # fp8 -240 pattern via u32 bitcast (4 elems/lane-cycle)
                    emit(nc.vector.memset(
                        pslab[32 * j:32 * (j + 1),
                              0:K * j * SEG].bitcast(mybir.dt.uint32),
                        0xF7F7F7F7))
                emit(nc.vector.memset(v1[:, 0:LEAD * W], NEG))
                emit(nc.vector.memset(v1[:, _cb(0):_cb(NCYC - 1) + 1:W], NEG))
                chain()
                emit(nc.vector.memset(v1[0:32, _cb(0):_cb(0) + 1], 0.0))

                dve_pass(1, v1, pslab, MLOG0, OP.add, OP.max, OP.max, OP.add)

                chain()
                emit(nc.vector.tensor_reduce(
                    out=atile[:],
                    in_=v1[:, _cb(0) + SEG:_cb(NCYC - 1) + SEG + 1:W],
                    axis=mybir.AxisListType.X, op=OP.max))
                marks["atile"] = sv
                vector.wait_ge(s_p, MM_BASE[2])
                chain()
                emit(nc.vector.tensor_tensor(out=d1[:], in0=atile[:],
                                             in1=bps[:], op=OP.subtract))
                chain()
                emit(nc.vector.scalar_tensor_tensor(
                    out=bias_t[:], in0=d1[:], scalar=-1.0 / SEG,
                    in1=cst[:, KH0:KH0 + 1], op0=OP.mult, op1=OP.subtract))
                marks["bias"] = sv

                emit(nc.vector.memset(v2[:, 0:LEAD * W], 0.0))
                emit(nc.vector.memset(v2[:, _cb(0):_cb(NCYC - 1) + 1:W], 0.0))
                chain()
                emit(nc.vector.memset(v2[0:32, _cb(0):_cb(0) + 1], 1.0))

                dve_pass(2, v2, phslab, MLIN0, OP.mult, OP.add, OP.add,
                         OP.mult)

                chain()
                cS1 = S - 1 + K * 3   # 114
                cS2 = S - 2 + K * 3   # 113
                emit(nc.vector.tensor_tensor(
                    out=vt[96:128],
                    in0=v2[96:128, _cb(cS2) + SEG:_cb(cS2) + SEG + 1],
                    in1=v2[96:128, _cb(cS1) + SEG:_cb(cS1) + SEG + 1],
                    op=OP.add))
                marks["vt"] = sv
                vector.wait_ge(s_a, 2 * NQUAD + 1)
                chain()
                emit(nc.vector.tensor_tensor(out=st[96:128], in0=lt[96:128],
                                             in1=atile[96:128], op=OP.add))
                marks["st"] = sv

            @block.tensor
            def _(tensor):
                def mms(p, vv, lhs):
                    for q in range(NQUAD):
                        tensor.wait_ge(s_v, scan_done[(p, QW * q + QW - 1)])
                        if q >= NBANK:
                            tensor.wait_ge(s_a,
                                           QUAD_BASE[p] + q - NBANK + 1)
                        elif p == 2:
                            tensor.wait_ge(s_a, NQUAD)
                        c0 = _cb(QW * q) + SEG
                        nc.tensor.matmul(
                            ph[q % NBANK][:], lhsT=lhs,
                            rhs=vv[:, c0:c0 + (QW - 1) * W + 1:W],
                            start=True, stop=True).then_inc(s_p, 1)

                tensor.wait_ge(s_dc, 16)
                mms(1, v1, cst[:, 0:128])
                tensor.wait_ge(s_v, marks["atile"])
                nc.tensor.matmul(bps[:], lhsT=cst[:, 0:128], rhs=atile[:],
                                 start=True, stop=True).then_inc(s_p, 1)
                tensor.wait_ge(s_dc, 32)
                mms(2, v2, permbt[:])

            @block.scalar
            def _(scalar):
                scalar.dma_start(cst[:], consts[:]).then_inc(s_dc, 16)
                scalar.wait_ge(s_dc, 16)
                scalar.dma_start(permbt[:], permb[:]).then_inc(s_dc, 16)
                # preload the ln+exp+copy activation table set
                nc.scalar.activation(out=lt[0:1], in_=cst[0:1, KH0:KH0 + 1],
                                     func=AF.Ln)
                nc.scalar.activation(out=st[0:1], in_=cst[0:1, KH0:KH0 + 1],
                                     func=AF.Exp)

                def copyq(p, vv, q):
                    scalar.wait_ge(s_p, MM_BASE[p] + q + 1)
                    dc = _cb(QW * q + K)
                    bank = ph[q % NBANK]
                    nc.scalar.activation(
                        out=vv[32:64, dc:dc + (QW - 1) * W + 1:W],
                        in_=bank[32:64, 0:QW], func=AF.Copy)
                    nc.scalar.activation(
                        out=vv[64:128, dc:dc + (QW - 1) * W + 1:W],
                        in_=bank[64:128, 0:QW],
                        func=AF.Copy).then_inc(s_a, 1)

                for q in range(NQUAD):
                    copyq(1, v1, q)
                scalar.wait_ge(s_v, marks["bias"])

                def expchunk(i):
                    c0, c1 = EB[i], EB[i + 1]
                    nc.scalar.activation(
                        out=phslab[:, c0 * SEG:c1 * SEG],
                        in_=pslab[:, c0 * SEG:c1 * SEG],
                        func=AF.Exp, bias=bias_t[:],
                        scale=1.0).then_inc(s_e, 1)

                nch = len(EB) - 1
                expchunk(0)
                expchunk(1)
                done_q = 0
                for i in range(2, nch):
                    # copies for quads gated by scans strictly before EB[i-1]
                    target = min(NQUAD, max(0, (EB[i - 1] - 1) // QW))
                    for q in range(done_q, target):
                        copyq(2, v2, q)
                    done_q = target
                    expchunk(i)
                for q in range(done_q, NQUAD):
                    copyq(2, v2, q)
                scalar.wait_ge(s_v, marks["vt"])
                nc.scalar.activation(out=lt[96:128], in_=vt[96:128],
                                     func=AF.Ln).then_inc(s_a, 1)
                scalar.wait_ge(s_v, marks["st"])
                nc.scalar.activation(out=lossT[96:128], in_=st[96:128],
                                     func=AF.Copy, scale=-1.0,
                                     bias=-KSUM).then_inc(s_a, 1)

            @block.gpsimd
            def _(gp):
                chunks = [jd(1, 6, 30), jd(2, 12, 36), jd(1, 30, 54),
                          jd(2, 36, 60), jd(1, 54, 78), jd(2, 60, 84),
                          jd(1, 78, NCYC), jd(2, 84, NCYC)]
                for i, (d, sr) in enumerate(chunks):
                    if i:
                        gp.wait_ge(s_dp, 16 * i)
                    gp.dma_start(d, sr).then_inc(s_dp, 16)

            @block.sync
            def _(sync):
                chunks = [jd(0, 0, 24), jd(3, 18, 42), jd(0, 24, 48),
                          jd(3, 42, 66), jd(0, 48, 72), jd(3, 66, 90),
                          jd(0, 72, 96), jd(3, 90, NCYC), jd(0, 96, NCYC)]
                for i, (d, sr) in enumerate(chunks):
                    if i:
                        sync.wait_ge(s_ds, 16 * i)
                    sync.dma_start(d, sr).then_inc(s_ds, 16)
                sync.wait_ge(s_a, 2 * NQUAD + 2)
                sync.dma_start(loss[:, :], lossT[96:128, :]).then_inc(s_o, 16)
                sync.wait_ge(s_o, 16)

    return nc


def host_prep(y_true, y_pred):
    import ml_dtypes
    y_true = np.asarray(y_true)
    y_pred = np.asarray(y_pred, dtype=np.float32)
    ext = np.full((B, S), BLANK, dtype=np.int64)
    ext[:, 1::2] = y_true.astype(np.int64)
    sh = np.concatenate([np.full((B, 2), -1, dtype=np.int64), ext[:, :-2]],
                        axis=1)
    allow = (ext != BLANK) & (ext != sh)          # [B, S]

    lq = np.log(y_pred + EPS).astype(np.float32)  # [B, T, C]

    permv = np.zeros((128, 128), dtype=np.float32)
    for kk in range(96):
        permv[kk, kk + 32] = 1.0
    khcol = np.zeros(128, np.float32)
    for j in range(NSEG):
        khcol[32 * j:32 * (j + 1)] = KHAT[j]

    in_maps = []
    for kcore in range(NCORES):
        bs = slice(kcore * BPC, (kcore + 1) * BPC)
        lqt = np.transpose(lq[bs], (0, 2, 1))     # [32, C, T]
        lpe = np.take_along_axis(
            lqt, ext[bs][:, :, None].astype(np.int64), axis=1)  # [32, S, T]
        mk = allow[bs]

        m = {}
        for j in range(NSEG):
            ncells = NCYC - K * j
            arr = np.full((BPC, ncells, SEG), -240.0, dtype=np.float32)
            arr[:, 0:S, :] = lpe[:, :, j * SEG:(j + 1) * SEG]
            m[f"pj{j}"] = (arr.reshape(BPC, ncells * SEG)
                           .astype(ml_dtypes.float8_e4m3))

        mlog = np.full((128, NCYC), NEG, dtype=np.float32)
        mlin = np.zeros((128, NCYC), dtype=np.float32)
        for j in range(NSEG):
            rows = slice(32 * j, 32 * (j + 1))
            for c in range(1, NCYC, 2):
                s = c - K * j
                if 0 <= s < S:
                    mlog[rows, c] = np.where(mk[:, s], 0.0, NEG)
                    mlin[rows, c] = mk[:, s].astype(np.float32)

        cstv = np.zeros((128, CW), np.float32)
        cstv[:, 0:128] = permv
        cstv[:, MLOG0:MLOG0 + NCYC] = mlog
        cstv[:, MLIN0:MLIN0 + NCYC] = mlin
        cstv[:, KH0] = khcol
        m["consts"] = cstv
        m["permb"] = permv.astype(ml_dtypes.bfloat16)
        in_maps.append(m)
    return in_maps


def _ensure_axon_devices():
    import jax
    try:
        devs = jax.devices()
        if len(devs) >= NCORES and all(d.platform != "cpu" for d in devs[:1]):
            return
    except Exception:
        pass
    try:
        jax.config.update("jax_platforms", None)
        jax.devices()
    except Exception:
        pass


def kernel(y_true, y_pred):
    _ensure_axon_devices()
    if "nc" not in _cache:
        _cache["nc"] = build_program()
    nc = _cache["nc"]
    in_maps = host_prep(y_true, y_pred)
    res = run_bass_kernel_spmd(nc, in_maps, list(range(NCORES)))
    out = np.concatenate([np.asarray(res.results[k]["loss"], dtype=np.float32)
                          for k in range(NCORES)], axis=0)
    return out.reshape(B, 1).astype(np.float32)
